# revision 1
# baseline (speedup 1.0000x reference)
"""Trainium2 Bass kernel for nn_CA_event (CA_event.forward batched ODE RHS).

reference:
    x   = state[:, 0:100]
    e_x = state[:, 100:200]
    W_a = state[:, 300:400]          (W_c = state[:, 200:300] unused)
    u   = W_a * (x + e_x - target)
    s   = x^2 / (1 + x^2)
    dx  = -x + s @ A.T + u * s
    out = concat([dx, -dx, 0, 0], axis=-1)      # [B, 400]

Strategy: pure data parallel over 8 NeuronCores (batch 131072 -> 16384
rows/core).  A [100,100] and target [100] are replicated.  Each core
streams its shard in 16 tiles of 1024 rows ([128 partitions x 8 rows]).
The device emits only the data-dependent half of the output (dx | -dx,
200 cols); the structurally-zero half (derivatives of W_c / W_a are
identically 0 for any input) is supplied host-side.

Math restructuring used on device (r := 1/(1+x^2) via one fused custom-DVE
op: bitwise-NOT Chebyshev seed + one Newton pass, ~1e-3 rel):
    rm1 = r - 1 = -s                                (1 VectorE op from x)
    u   = W_a * (x + e - tgt)                       (3 VectorE ops)
    t   = rm1 * u = -u*s                            (1 VectorE op)
    PSUM = I@x + I@t + rm1@A.T = x - u*s - s@A.T = -dx
          (identity-matmul accumulation on TensorE; per-128-row-group
           PE transpose of rm1 feeds the A.T matmul)
    -dx -> out[:,100:200], dx = -(-dx) -> out[:,0:100] (ScalarE, from PSUM)

DMA: loads via SWDGE (GpSimd ring), stores via the SP HWDGE ring --
separate issue paths; reads skip the unused W_c columns.
"""

import os
import sys

try:
    import concourse  # noqa: F401  (resolves via the environment's default path)
except ImportError:  # fall back for bare environments
    sys.path.insert(0, "/opt/trn_rl_repo")

import numpy as np

import concourse.bass as bass
import concourse.bacc as bacc
import concourse.mybir as mybir
from concourse import tile
from concourse import masks

DIM = 100
BATCH = 131072
NCORES = 8
ROWS_PER_CORE = BATCH // NCORES          # 16384
R = 8                                    # batch rows per partition per tile
TILE_ROWS = 128 * R                      # 1024
NTILES = ROWS_PER_CORE // TILE_ROWS      # 16

F32 = mybir.dt.float32

_RUNNERS = {}  # key -> runner dict
_CA_OPS = None


def _register_ca_ops():
    """Register two fused custom-DVE ops computing r-1 = 1/(1+x^2) - 1 from x.

    CA_RECIP_SEED: in0=x -> y1   (Chebyshev bitwise-NOT seed + 1 NR pass)
    CA_RECIP_FIN:  in0=x, in1=y1 -> (r - 1)   (second NR pass, then -1)

    Same math/constants as dve_ops.RECIPROCAL_APPROX_FAST (~51 ULP), with the
    (1 + x^2) denominator computation folded into both ops and the final -1
    folded into the second op.  Registered at runtime (appended to
    dve_ops.OPS) so no repo files change; the per-NEFF DVE table generator
    resolves ops by name from that list in-process.
    """
    global _CA_OPS
    if _CA_OPS is not None:
        return _CA_OPS
    from concourse import dve_ops
    from concourse.dve_spec import Spec, Src0, Src1, C0, C1, One, Bin, AluOp, sq
    from concourse.dve_uop import DveOpSpec

    c = dve_ops.RECIP_APPROX_FAST_CONSTS  # s0 (cheby scale), s1 (cheby 2), imm2=2.0

    # ---- op A: y1 = seed + one NR pass, d = 1 + x^2 ----
    dA = sq(Src0) + One
    ndA = Bin(AluOp.BITWISE_NOT, dA, dA)
    y0 = ndA * C0
    bodyA = y0 * (C1 - dA * y0)

    def refA(in0, in1, s0, s1, imm2):
        d = (1.0 + in0.astype(np.float32) * in0).astype(np.float32)
        nd = (~d.view(np.int32)).view(np.float32)
        yy0 = (nd * np.float32(s0)).astype(np.float32)
        return (yy0 * (np.float32(s1) - d * yy0)).astype(np.float32)

    # ---- op B: out = y1*(2 - d*y1) - 1  (= r - 1) ----
    dB = sq(Src0) + One
    bodyB = Src1 * (C0 - dB * Src1) - One

    def refB(in0, in1, s0, s1, imm2):
        d = (1.0 + in0.astype(np.float32) * in0).astype(np.float32)
        return (in1 * (np.float32(s0) - d * in1) - 1.0).astype(np.float32)

    # ---- op C: one-shot rm1 = seed + single NR - 1 (lower accuracy ~1e-3) --
    dC = sq(Src0) + One
    ndC = Bin(AluOp.BITWISE_NOT, dC, dC)
    y0C = ndC * C0
    bodyC = y0C * (C1 - dC * y0C) - One

    def refC(in0, in1, s0, s1, imm2):
        d = (1.0 + in0.astype(np.float32) * in0).astype(np.float32)
        nd = (~d.view(np.int32)).view(np.float32)
        yy0 = (nd * np.float32(s0)).astype(np.float32)
        return (yy0 * (np.float32(s1) - d * yy0) - 1.0).astype(np.float32)

    specs = [
        ("CA_RECIP_SEED", Spec(body=bodyA, reference=refA)),
        ("CA_RECIP_FIN", Spec(body=bodyB, reference=refB)),
        ("CA_RM1_NR1", Spec(body=bodyC, reference=refC)),
    ]
    ops = []
    for name, spec in specs:
        if name not in dve_ops._SUB_OPCODE_FOR_NAME:
            row = max(dve_ops._SUB_OPCODE_FOR_NAME.values()) + 1
            assert row < 0x20
            dve_ops._SUB_OPCODE_FOR_NAME[name] = row
        shas = {}
        for ver in ("v3", "v4"):
            s = DveOpSpec(
                name=name,
                opcode=dve_ops.get_dve_sub_opcode(name),
                uops=dve_ops.lower(spec, ver=ver),
                rd1_en=dve_ops.has_src1(spec),
            )
            shas[ver] = s.sha(ver)
        op = dve_ops.DveOp(name, spec, subdim=False, uops_sha=shas)
        if not any(o.name == name for o in dve_ops.OPS):
            dve_ops.OPS.append(op)
            dve_ops.CUSTOM_DVE_SPECS[name] = spec
        ops.append(op)
    _CA_OPS = tuple(ops)
    return _CA_OPS


def _build(repeat=1, ablate=(), read_cols=300, use_custom=True, pool_offload=False,
           loop_k=1, pe_accum=True, store_act=False, dma_balance=False, nr1=True,
           pool_he=False, swdge_load=True, r_rows=8, pair_dma=True, dma_group=2):
    """Build the per-core Bacc module.

    ablate: stages to skip for timing experiments only (output wrong):
            'dve', 'pe', 'act', 'load', 'store'
    read_cols: 300 (two DMAs, skip W_c) or 400 (one fully-contiguous DMA)
    use_custom: fused custom-DVE recip ops vs stock op chain
    pool_offload: x+e and -target adds on GpSimd instead of VectorE
    loop_k: hardware For_i repetitions of the whole pass (timing; idempotent)
    pe_accum: accumulate +x and +t into the matmul PSUM via identity matmuls
              (f32r moving, 4 groups per matmul) instead of DVE adds
    """
    ablate = set(ablate)
    R = r_rows                      # shadow the module default per-build
    NTILES = ROWS_PER_CORE // (128 * R)
    F32R = mybir.dt.float32r
    nc = bacc.Bacc("TRN2", target_bir_lowering=False, debug=False)

    state = nc.declare_dram_parameter("state", [ROWS_PER_CORE, 4 * DIM], F32, isOutput=False)
    A = nc.declare_dram_parameter("A", [DIM, DIM], F32, isOutput=False)
    target = nc.declare_dram_parameter("target", [DIM], F32, isOutput=False)
    out = nc.declare_dram_parameter("out", [ROWS_PER_CORE, 2 * DIM], F32, isOutput=True)

    state_4d = state.ap().rearrange("(t p r) c -> t p r c", p=128, r=R)
    out_t = out.ap().rearrange("(t p r) c -> t p (r c)", p=128, r=R)
    G = dma_group
    state_4dp = state.ap().rearrange("(t p r) c -> t p r c", p=128, r=G * R)
    out_tp = out.ap().rearrange("(t p r) c -> t p (r c)", p=128, r=G * R)

    if use_custom:
        op_seed, op_fin, op_nr1 = _register_ca_ops()

    eng_he = "pool" if pool_offload else "dve"

    # SBUF budget: shrink buffer counts for bigger tiles
    inp_b = 4 if R <= 8 else 2
    work_b = 4 if R <= 8 else 2
    outp_b = 4 if R <= 8 else 3
    if pair_dma:
        inp_b = 4 if G == 2 else 2
        outp_b = 3 if G == 2 else 2
    with tile.TileContext(nc) as tc:
        with (
            tc.tile_pool(name="consts", bufs=1) as consts,
            tc.tile_pool(name="inp", bufs=inp_b) as inp,
            tc.tile_pool(name="work", bufs=work_b) as work,
            tc.tile_pool(name="outp", bufs=outp_b) as outp,
            tc.tile_pool(name="sT", bufs=6) as sT_pool,
            tc.tile_pool(name="psum_t", bufs=4, space="PSUM") as psum_t,
            tc.tile_pool(name="psum_mm", bufs=4, space="PSUM") as psum_mm_pool,
        ):
            # ---- one-time constants -------------------------------------
            identity = consts.tile([128, 128], F32)
            masks.make_identity(nc, identity[:])

            a_sb = consts.tile([DIM, DIM], F32)
            nc.sync.dma_start(out=a_sb[:], in_=A.ap())

            # A^T in SBUF (rhs for the per-group matmuls)
            a_ps = psum_t.tile([DIM, DIM], F32, tag="tr")
            nc.tensor.transpose(a_ps[:], a_sb[:], identity[:DIM, :DIM])
            at_sb = consts.tile([DIM, DIM], F32)
            nc.scalar.copy(at_sb[:], a_ps[:])

            # target broadcast to [128, R, 100]
            t_row = consts.tile([1, DIM], F32)
            nc.sync.dma_start(out=t_row[:], in_=target.ap()[None, :])
            t_bc = consts.tile([128, DIM], F32)
            nc.gpsimd.partition_broadcast(t_bc[:], t_row[:])
            tgtb = consts.tile([128, R, DIM], F32)
            for g in range(R):
                nc.scalar.copy(tgtb[:, g, :], t_bc[:])

            # ---- main loop ----------------------------------------------
            def emit_pass():
                pair = {}
                for i in range(NTILES):
                    # loads on the SP HWDGE ring, stores (+W_a load when
                    # balancing) on the ACT ring
                    if dma_balance:
                        ring_a = nc.sync if i % 2 == 0 else nc.scalar
                        ring_b = nc.scalar if i % 2 == 0 else nc.sync
                        w_ring = ring_b
                    else:
                        ring_a = nc.gpsimd if swdge_load else nc.sync
                        ring_b = nc.scalar if store_act else nc.sync
                        w_ring = nc.gpsimd if swdge_load else nc.sync
                    if pair_dma and read_cols == 300:
                        # one load/store DMA per PAIR of compute tiles
                        # (2x transfer size -> better DMA efficiency)
                        if i % G == 0:
                            pair["in"] = inp.tile([128, G * R, 2 * DIM], F32, tag="in", name="pin")
                            pair["w"] = inp.tile([128, G * R, DIM], F32, tag="inw", name="pw")
                            if "load" not in ablate:
                                ring_a.dma_start(out=pair["in"][:],
                                                 in_=state_4dp[i // G, :, :, 0:2 * DIM])
                                w_ring.dma_start(out=pair["w"][:],
                                                 in_=state_4dp[i // G, :, :, 3 * DIM:4 * DIM])
                            pair["out"] = outp.tile([128, G * R, 2 * DIM], F32, tag="out", name="pout")
                        hs = slice((i % G) * R, (i % G) * R + R)
                        x = pair["in"][:, hs, 0:DIM]
                        e = pair["in"][:, hs, DIM:2 * DIM]
                        w = pair["w"][:, hs, :]
                    elif read_cols == 400:
                        in_tile = inp.tile([128, R, 4 * DIM], F32, tag="in")
                        if "load" not in ablate:
                            ring_a.dma_start(out=in_tile[:], in_=state_4d[i])
                        x = in_tile[:, :, 0:DIM]
                        e = in_tile[:, :, DIM:2 * DIM]
                        w = in_tile[:, :, 3 * DIM:4 * DIM]
                    else:
                        in_tile = inp.tile([128, R, 2 * DIM], F32, tag="in")
                        w_tile = inp.tile([128, R, DIM], F32, tag="inw")
                        if "load" not in ablate:
                            ring_a.dma_start(out=in_tile[:], in_=state_4d[i, :, :, 0:2 * DIM])
                            w_ring.dma_start(out=w_tile[:], in_=state_4d[i, :, :, 3 * DIM:4 * DIM])
                        x = in_tile[:, :, 0:DIM]
                        e = in_tile[:, :, DIM:2 * DIM]
                        w = w_tile[:]

                    skip_dve = "dve" in ablate

                    # he = x + e ; hm = he - target   (GpSimd when offloaded)
                    he = work.tile([128, R, DIM], F32, tag="he")
                    hm = work.tile([128, R, DIM], F32, tag="hm")
                    if not skip_dve:
                        if eng_he == "pool":
                            nc.gpsimd.tensor_add(he[:], x, e)
                            nc.gpsimd.tensor_sub(hm[:], he[:], tgtb[:])
                        elif pool_he:
                            nc.gpsimd.tensor_add(he[:], x, e)
                            nc.vector.tensor_sub(hm[:], he[:], tgtb[:])
                        else:
                            nc.vector.tensor_add(he[:], x, e)
                            nc.vector.tensor_sub(hm[:], he[:], tgtb[:])

                    # rm1 = 1/(1+x^2) - 1  (= -s)
                    rm1 = work.tile([128, R, DIM], F32, tag="rm1")
                    if not skip_dve:
                        if use_custom and nr1:
                            nc.vector._custom_dve(
                                op_nr1, out=rm1[:], in0=x,
                                s0=float(np.float32(-0.23549792)),
                                s1=float(np.float32(2.0017324)),
                            )
                        elif use_custom:
                            y1 = work.tile([128, R, DIM], F32, tag="y1")
                            nc.vector._custom_dve(
                                op_seed, out=y1[:], in0=x,
                                s0=float(np.float32(-0.23549792)),
                                s1=float(np.float32(2.0017324)),
                            )
                            nc.vector._custom_dve(
                                op_fin, out=rm1[:], in0=x, in1=y1[:], s0=2.0,
                            )
                        else:
                            xx = work.tile([128, R, DIM], F32, tag="xx")
                            nc.scalar.square(xx[:], x)
                            d = work.tile([128, R, DIM], F32, tag="d")
                            nc.vector.tensor_scalar_add(d[:], xx[:], 1.0)
                            rr = work.tile([128, R, DIM], F32, tag="rr")
                            nc.vector.reciprocal_approx_fast(out=rr[:], in_=d[:])
                            nc.vector.tensor_scalar_add(rm1[:], rr[:], -1.0)

                    u = work.tile([128, R, DIM], F32, tag="u")
                    t = work.tile([128, R, DIM], F32, tag="t")
                    if not skip_dve:
                        nc.vector.tensor_mul(u[:], hm[:], w)
                        nc.vector.tensor_mul(t[:], rm1[:], u[:])   # -u*s
                    else:
                        nc.vector.tensor_copy(rm1[:], x)
                        nc.vector.tensor_copy(t[:], x)

                    use_pe_accum = pe_accum and "pe" not in ablate and not skip_dve
                    if pair_dma and read_cols == 300:
                        out_tile = pair["out"][:, slice((i % G) * R, (i % G) * R + R), :]
                    else:
                        out_tile = outp.tile([128, R, 2 * DIM], F32, tag="out")

                    if use_pe_accum:
                        # psum := x + t  (identity matmuls, 4 groups = one
                        # 1-bank psum half per matmul), then += rm1[g] @ A.T
                        # per group -> psum = x - u*s - s@A.T = -dx
                        for h in range(R // 4):
                            mmh = psum_mm_pool.tile([128, 4, 128], F32, tag="mm")
                            gs = slice(4 * h, 4 * h + 4)
                            nc.tensor.matmul(mmh[:, :, 0:DIM], identity[:],
                                             x[:, gs, :],
                                             start=True, stop=False,
                                             skip_group_check=True)
                            nc.tensor.matmul(mmh[:, :, 0:DIM], identity[:],
                                             t[:, gs, :],
                                             start=False, stop=False,
                                             skip_group_check=True)
                            for j in range(4):
                                g = 4 * h + j
                                ps_tr = psum_t.tile([DIM, 128], F32, tag="tr")
                                nc.tensor.transpose(ps_tr[:], rm1[:, g, :], identity[:])
                                st_sb = sT_pool.tile([DIM, 128], F32, tag="st")
                                nc.scalar.copy(st_sb[:], ps_tr[:])
                                nc.tensor.matmul(mmh[:, j, 0:DIM], st_sb[:], at_sb[:],
                                                 start=False, stop=True,
                                                 skip_group_check=True)
                            # -dx -> cols 100:200 (ScalarE copy from PSUM);
                            # dx -> cols 0:100
                            nc.scalar.copy(out_tile[:, gs, DIM:2 * DIM], mmh[:, :, 0:DIM])
                            nc.scalar.mul(out_tile[:, gs, 0:DIM], mmh[:, :, 0:DIM], -1.0)
                    else:
                        mm = psum_mm_pool.tile([128, R, 128], F32, tag="mmf", bufs=2)
                        q = work.tile([128, R, DIM], F32, tag="q")
                        if not skip_dve:
                            nc.vector.tensor_add(q[:], t[:], x)    # x - u*s
                        else:
                            nc.vector.tensor_copy(q[:], x)
                        if "pe" not in ablate:
                            for g in range(R):
                                ps_tr = psum_t.tile([DIM, 128], F32, tag="tr")
                                nc.tensor.transpose(ps_tr[:], rm1[:, g, :], identity[:])
                                st_sb = sT_pool.tile([DIM, 128], F32, tag="st")
                                nc.scalar.copy(st_sb[:], ps_tr[:])
                                nc.tensor.matmul(mm[:, g, 0:DIM], st_sb[:], at_sb[:],
                                                 start=True, stop=True)
                            nc.vector.tensor_add(out_tile[:, :, DIM:2 * DIM], q[:], mm[:, :, 0:DIM])
                        else:
                            nc.vector.tensor_add(out_tile[:, :, DIM:2 * DIM], q[:], q[:])
                        if "act" not in ablate:
                            nc.scalar.mul(out_tile[:, :, 0:DIM], out_tile[:, :, DIM:2 * DIM], -1.0)
                        else:
                            nc.vector.tensor_copy(out_tile[:, :, 0:DIM], out_tile[:, :, DIM:2 * DIM])
                    if "store" not in ablate:
                        if pair_dma and read_cols == 300:
                            if i % G == G - 1:
                                ring_b.dma_start(out=out_tp[i // G], in_=pair["out"][:])
                        else:
                            ring_b.dma_start(out=out_t[i], in_=out_tile[:])

            if loop_k > 1:
                stag = bool(int(os.environ.get("CA_STAG", "0")))
                with tc.For_i(0, loop_k, 1, staggered_reset=stag):
                    emit_pass()
            else:
                for _ in range(repeat):
                    emit_pass()

    nc.compile()
    return nc


def _make_runner(nc):
    """Cached jitted shard_map executor for a prebuilt Bacc module.

    Mirrors bass2jax.run_bass_via_pjrt, but keeps the jitted callable (and
    device-resident inputs) reusable across calls so repeated invocations
    don't re-trace/re-compile.
    """
    import jax
    from jax.experimental.shard_map import shard_map
    from jax.sharding import Mesh, PartitionSpec
    from concourse import bass2jax

    bass2jax.install_neuronx_cc_hook()

    partition_name = nc.partition_id_tensor.name if nc.partition_id_tensor else None
    in_names, out_names, out_avals, zero_shapes = [], [], [], []
    for alloc in nc.m.functions[0].allocations:
        if not isinstance(alloc, mybir.MemoryLocationSet):
            continue
        name = alloc.memorylocations[0].name
        if alloc.kind == "ExternalInput":
            if name != partition_name:
                in_names.append(name)
        elif alloc.kind == "ExternalOutput":
            out_names.append(name)
            shape = tuple(alloc.tensor_shape)
            dtype = mybir.dt.np(alloc.dtype)
            out_avals.append(jax.core.ShapedArray(shape, dtype))
            zero_shapes.append((shape, dtype))
    n_params = len(in_names)
    n_outs = len(out_names)
    bind_in_names = list(in_names) + list(out_names)
    if partition_name is not None:
        bind_in_names.append(partition_name)

    donate = tuple(range(n_params, n_params + n_outs))

    def _body(*args):
        operands = list(args)
        if partition_name is not None:
            operands.append(bass2jax.partition_id_tensor())
        outs = bass2jax._bass_exec_p.bind(
            *operands,
            out_avals=tuple(out_avals),
            in_names=tuple(bind_in_names),
            out_names=tuple(out_names),
            lowering_input_output_aliases=(),
            sim_require_finite=True,
            sim_require_nnan=True,
            nc=nc,
        )
        return tuple(outs)

    devices = jax.devices()[:NCORES]
    assert len(devices) == NCORES
    mesh = Mesh(np.asarray(devices), ("core",))
    in_specs = (PartitionSpec("core"),) * (n_params + n_outs)
    out_specs = (PartitionSpec("core"),) * n_outs
    # No donation: the kernel writes every element of every output, so the
    # zero "out" operands are never read (they exist only to satisfy the NEFF
    # operand list) and can be reused across calls.
    del donate
    sharded = jax.jit(
        shard_map(_body, mesh=mesh, in_specs=in_specs, out_specs=out_specs,
                  check_rep=False),
        keep_unused=True,
    )

    return {
        "fn": sharded,
        "mesh": mesh,
        "in_names": in_names,
        "out_names": out_names,
        "zero_shapes": zero_shapes,
        "n_params": n_params,
    }


def _get_runner(repeat=1, **buildkw):
    key = (repeat, tuple(sorted(buildkw.items())))
    if key not in _RUNNERS:
        _RUNNERS[key] = _make_runner(_build(repeat, **buildkw))
    return _RUNNERS[key]


def _concat_inputs(state, A, target):
    return {
        "state": state.reshape(NCORES * ROWS_PER_CORE, 4 * DIM),
        "A": np.concatenate([A] * NCORES, axis=0),
        "target": np.concatenate([target] * NCORES, axis=0),
    }


def run_on_device(state, A, target, repeat=1, n_timed=0, **buildkw):
    """Execute; optionally time n_timed extra calls (device-resident inputs).

    Returns (out_global [8*16384, 200], times_s list).
    """
    import jax
    import jax.numpy as jnp
    from jax.sharding import NamedSharding, PartitionSpec
    import time

    runner = _get_runner(repeat, **buildkw)
    fn = runner["fn"]
    mesh = runner["mesh"]
    shard = NamedSharding(mesh, PartitionSpec("core"))

    cat = _concat_inputs(state, A, target)
    dev_in = [jax.device_put(cat[name], shard) for name in runner["in_names"]]
    dev_z = [
        jax.device_put(np.zeros((NCORES * sh[0], *sh[1:]), dt), shard)
        for (sh, dt) in runner["zero_shapes"]
    ]
    jax.block_until_ready(dev_z)

    outs = fn(*dev_in, *dev_z)
    jax.block_until_ready(outs)
    times = []
    for _ in range(n_timed):
        t0 = time.perf_counter()
        o = fn(*dev_in, *dev_z)
        jax.block_until_ready(o)
        times.append(time.perf_counter() - t0)
    result = np.asarray(outs[0])
    return result, times


def kernel(state, A, target):
    state = np.ascontiguousarray(np.asarray(state, dtype=np.float32))
    A = np.ascontiguousarray(np.asarray(A, dtype=np.float32))
    target = np.ascontiguousarray(np.asarray(target, dtype=np.float32))
    assert state.shape == (BATCH, 4 * DIM)

    half, _ = run_on_device(state, A, target, repeat=1)
    full = np.zeros((BATCH, 4 * DIM), dtype=np.float32)
    full[:, :2 * DIM] = half
    return full



# revision 2
# speedup vs baseline: 1.6682x; 1.6682x over previous
"""Trainium2 Bass kernel for nn_CA_event (CA_event.forward batched ODE RHS).

reference:
    x   = state[:, 0:100]
    e_x = state[:, 100:200]
    W_a = state[:, 300:400]          (W_c = state[:, 200:300] unused)
    u   = W_a * (x + e_x - target)
    s   = x^2 / (1 + x^2)
    dx  = -x + s @ A.T + u * s
    out = concat([dx, -dx, 0, 0], axis=-1)      # [B, 400]

Strategy: pure data parallel over 8 NeuronCores (batch 131072 -> 16384
rows/core), FEATURE-MAJOR bf16 layout.  Host packs the three used state
slices transposed into one bf16 tensor xew[c] = [3, 100, 16384]
(planes x / e_x / W_a; feature dim on partitions), sends the constants
nAT = -A.T (bf16) and ntgt = -target ([100,1] f32).  The rel-err gate is
2e-2; bf16 I/O keeps the L2 error ~3e-3 while cutting HBM traffic from
2000 B/row (f32, both dx and -dx stored) to 800 B/row: reads 600 B
(x,e,W_a bf16) + writes 200 B (dx bf16 only; -dx is mirrored host-side,
the W_c/W_a derivative halves are structurally zero for any input).

Per 2048-row tile [100 partitions x 2048]:
    V:   he2 = x + e                     (bf16 tensor_tensor, 2x mode)
         rm1 = 1/(1+x^2) - 1 = -s       (custom DVE op, 1x: NOT-seed + 1 NR)
         u   = hm * w                   (2x)
         t2  = u * rm1 = -u*s           (2x)
    ACT: hm  = he2 + (-target)          (Identity activation, per-partition bias)
         out = Copy(psum) -> bf16       (the only PSUM read)
    PE:  psum = nAT@rm1 + nI@t2 + nI@x = A@s + u*s - x = dx^T
         (512-col chunks; no transposes anywhere -- the feature-major
          layout makes the moving operands k-major natively)
    DMA: one packed 3-plane load (sync/SP HWDGE), store on GpSimd SWDGE.
"""

import os
import sys

try:
    import concourse  # noqa: F401  (resolves via the environment's default path)
except ImportError:  # fall back for bare environments
    sys.path.insert(0, "/opt/trn_rl_repo")

import numpy as np
import ml_dtypes

import concourse.bass as bass
import concourse.bacc as bacc
import concourse.mybir as mybir
from concourse import tile
from concourse import masks

DIM = 100
BATCH = 131072
NCORES = 8
ROWS_PER_CORE = BATCH // NCORES          # 16384

F32 = mybir.dt.float32
BF16 = mybir.dt.bfloat16
NP_BF16 = ml_dtypes.bfloat16

_RUNNERS = {}  # key -> runner dict
_CA_OPS = None


def _register_ca_ops():
    """Register the fused custom-DVE op rm1 = 1/(1+x^2) - 1 (= -s) from x.

    Chebyshev bitwise-NOT reciprocal seed + one Newton pass on d = 1+x^2,
    minus 1; ~1e-3 rel accuracy on r, which is plenty under the 2e-2 gate.
    Same body as the baseline's proven CA_RM1_NR1.
    """
    global _CA_OPS
    if _CA_OPS is not None:
        return _CA_OPS
    from concourse import dve_ops
    from concourse.dve_spec import Spec, Src0, C0, C1, One, Bin, AluOp, sq
    from concourse.dve_uop import DveOpSpec

    dC = sq(Src0) + One
    ndC = Bin(AluOp.BITWISE_NOT, dC, dC)
    y0C = ndC * C0
    bodyC = y0C * (C1 - dC * y0C) - One

    def refC(in0, in1, s0, s1, imm2):
        d = (1.0 + in0.astype(np.float32) * in0).astype(np.float32)
        nd = (~d.view(np.int32)).view(np.float32)
        yy0 = (nd * np.float32(s0)).astype(np.float32)
        return (yy0 * (np.float32(s1) - d * yy0) - 1.0).astype(np.float32)

    name, spec = "CA_RM1_NR1", Spec(body=bodyC, reference=refC)
    if name not in dve_ops._SUB_OPCODE_FOR_NAME:
        row = max(dve_ops._SUB_OPCODE_FOR_NAME.values()) + 1
        assert row < 0x20
        dve_ops._SUB_OPCODE_FOR_NAME[name] = row
    shas = {}
    for ver in ("v3", "v4"):
        s = DveOpSpec(
            name=name,
            opcode=dve_ops.get_dve_sub_opcode(name),
            uops=dve_ops.lower(spec, ver=ver),
            rd1_en=dve_ops.has_src1(spec),
        )
        shas[ver] = s.sha(ver)
    op = dve_ops.DveOp(name, spec, subdim=False, uops_sha=shas)
    if not any(o.name == name for o in dve_ops.OPS):
        dve_ops.OPS.append(op)
        dve_ops.CUSTOM_DVE_SPECS[name] = spec
    _CA_OPS = (op,)
    return _CA_OPS


def _build(repeat=1, loop_k=1, t_rows=2048, he_eng="vector", hm_eng="act",
           store_q="gpsimd", load_q="sync", ablate=()):
    """Build the per-core Bacc module.

    he_eng: 'vector' (DVE tensor_add) or 'pe' (identity-matmul accumulate +
            ACT bias from PSUM)
    hm_eng: 'act' (Identity activation w/ per-partition bias) or 'vector'
            (needs a broadcast tgt tile; only for experiments)
    ablate: stages to skip for timing experiments only (output wrong):
            'dve', 'pe', 'act', 'load', 'store'
    """
    ablate = set(ablate)
    T = t_rows
    NTILES = ROWS_PER_CORE // T
    NCH = T // 512                     # matmul chunks per tile
    nc = bacc.Bacc("TRN2", target_bir_lowering=False, debug=False)

    xew = nc.declare_dram_parameter("xew", [3 * DIM, ROWS_PER_CORE], BF16, isOutput=False)
    nAT = nc.declare_dram_parameter("nAT", [DIM, DIM], BF16, isOutput=False)
    ntgt = nc.declare_dram_parameter("ntgt", [DIM, 1], F32, isOutput=False)
    out = nc.declare_dram_parameter("out", [DIM, ROWS_PER_CORE], BF16, isOutput=True)

    # [t][f, c, w]: feature f on partitions, plane c (x/e/w), row window w
    xew_t = xew.ap().rearrange("(c f) (t w) -> t f c w", c=3, w=T)
    out_t = out.ap().rearrange("f (t w) -> t f w", w=T)

    (op_rm1,) = _register_ca_ops()

    with tile.TileContext(nc) as tc:
        with (
            tc.tile_pool(name="consts", bufs=1) as consts,
            tc.tile_pool(name="inp", bufs=3) as inp,
            tc.tile_pool(name="work", bufs=3) as work,
            tc.tile_pool(name="outp", bufs=3) as outp,
            tc.tile_pool(name="psum_mm", bufs=2, space="PSUM") as psum_mm,
        ):
            # ---- one-time constants -------------------------------------
            nat_sb = consts.tile([DIM, DIM], BF16)
            nc.sync.dma_start(out=nat_sb[:], in_=nAT.ap())

            tgt_sb = consts.tile([DIM, 1], F32)
            nc.sync.dma_start(out=tgt_sb[:], in_=ntgt.ap())

            ident = consts.tile([DIM, DIM], F32)
            masks.make_identity(nc, ident[:])
            ni_sb = consts.tile([DIM, DIM], BF16)
            nc.scalar.mul(ni_sb[:], ident[:], -1.0)

            # ---- main loop ----------------------------------------------
            def emit_pass():
                for i in range(NTILES):
                    in_tile = inp.tile([DIM, 3, T], BF16, tag="in")
                    if "load" not in ablate:
                        lq = nc.sync if load_q == "sync" else nc.scalar
                        lq.dma_start(out=in_tile[:], in_=xew_t[i])
                    x = in_tile[:, 0, :]
                    e = in_tile[:, 1, :]
                    w = in_tile[:, 2, :]

                    skip_dve = "dve" in ablate

                    he2 = work.tile([DIM, T], BF16, tag="he")
                    hm = work.tile([DIM, T], BF16, tag="hm")
                    u = work.tile([DIM, T], BF16, tag="u")
                    rm1 = work.tile([DIM, T], BF16, tag="rm1")
                    t2 = work.tile([DIM, T], BF16, tag="t2")
                    if not skip_dve:
                        nc.vector.tensor_add(he2[:], x, e)
                        if hm_eng == "act":
                            nc.scalar.add(hm[:], he2[:], tgt_sb[:, 0:1])
                        else:
                            nc.vector.scalar_tensor_tensor(
                                hm[:], he2[:], tgt_sb[:, 0:1], he2[:],
                                op0=mybir.AluOpType.add,
                                op1=mybir.AluOpType.bypass,
                            )
                        nc.vector.tensor_mul(u[:], hm[:], w)
                        nc.vector._custom_dve(
                            op_rm1, out=rm1[:], in0=x,
                            s0=float(np.float32(-0.23549792)),
                            s1=float(np.float32(2.0017324)),
                        )
                        nc.vector.tensor_mul(t2[:], u[:], rm1[:])
                    else:
                        nc.vector.tensor_copy(rm1[:], x)
                        nc.vector.tensor_copy(t2[:], x)

                    mm = psum_mm.tile([DIM, T], F32, tag="mm")
                    if "pe" not in ablate:
                        for ci in range(NCH):
                            cs = slice(512 * ci, 512 * (ci + 1))
                            nc.tensor.matmul(mm[:, cs], nat_sb[:], rm1[:, cs],
                                             start=True, stop=False)
                            nc.tensor.matmul(mm[:, cs], ni_sb[:], t2[:, cs],
                                             start=False, stop=False)
                            nc.tensor.matmul(mm[:, cs], ni_sb[:], x[:, cs],
                                             start=False, stop=True)
                    else:
                        nc.vector.tensor_copy(mm[:], t2[:])

                    out_tile = outp.tile([DIM, T], BF16, tag="out")
                    if "act" not in ablate:
                        nc.scalar.copy(out_tile[:], mm[:])
                    else:
                        nc.vector.tensor_copy(out_tile[:], rm1[:])

                    if "store" not in ablate:
                        sq_ = {"gpsimd": nc.gpsimd, "scalar": nc.scalar,
                               "sync": nc.sync}[store_q]
                        sq_.dma_start(out=out_t[i], in_=out_tile[:])

            if loop_k > 1:
                stag = bool(int(os.environ.get("CA_STAG", "0")))
                with tc.For_i(0, loop_k, 1, staggered_reset=stag):
                    emit_pass()
            else:
                for _ in range(repeat):
                    emit_pass()

    nc.compile()
    return nc


def _make_runner(nc):
    """Cached jitted shard_map executor for a prebuilt Bacc module."""
    import jax
    from jax.experimental.shard_map import shard_map
    from jax.sharding import Mesh, PartitionSpec
    from concourse import bass2jax

    bass2jax.install_neuronx_cc_hook()

    partition_name = nc.partition_id_tensor.name if nc.partition_id_tensor else None
    in_names, out_names, out_avals, zero_shapes = [], [], [], []
    for alloc in nc.m.functions[0].allocations:
        if not isinstance(alloc, mybir.MemoryLocationSet):
            continue
        name = alloc.memorylocations[0].name
        if alloc.kind == "ExternalInput":
            if name != partition_name:
                in_names.append(name)
        elif alloc.kind == "ExternalOutput":
            out_names.append(name)
            shape = tuple(alloc.tensor_shape)
            dtype = mybir.dt.np(alloc.dtype)
            out_avals.append(jax.core.ShapedArray(shape, dtype))
            zero_shapes.append((shape, dtype))
    n_params = len(in_names)
    n_outs = len(out_names)
    bind_in_names = list(in_names) + list(out_names)
    if partition_name is not None:
        bind_in_names.append(partition_name)

    def _body(*args):
        operands = list(args)
        if partition_name is not None:
            operands.append(bass2jax.partition_id_tensor())
        outs = bass2jax._bass_exec_p.bind(
            *operands,
            out_avals=tuple(out_avals),
            in_names=tuple(bind_in_names),
            out_names=tuple(out_names),
            lowering_input_output_aliases=(),
            sim_require_finite=True,
            sim_require_nnan=True,
            nc=nc,
        )
        return tuple(outs)

    devices = jax.devices()[:NCORES]
    assert len(devices) == NCORES
    mesh = Mesh(np.asarray(devices), ("core",))
    in_specs = (PartitionSpec("core"),) * (n_params + n_outs)
    out_specs = (PartitionSpec("core"),) * n_outs
    sharded = jax.jit(
        shard_map(_body, mesh=mesh, in_specs=in_specs, out_specs=out_specs,
                  check_rep=False),
        keep_unused=True,
    )

    return {
        "fn": sharded,
        "mesh": mesh,
        "in_names": in_names,
        "out_names": out_names,
        "zero_shapes": zero_shapes,
        "n_params": n_params,
    }


def _get_runner(repeat=1, **buildkw):
    key = (repeat, tuple(sorted(buildkw.items())))
    if key not in _RUNNERS:
        _RUNNERS[key] = _make_runner(_build(repeat, **buildkw))
    return _RUNNERS[key]


def _concat_inputs(state, A, target):
    """Host-side pack: shard + transpose to feature-major bf16.

    xew[c] = [x_c^T; e_c^T; w_c^T] stacked as [3*100, 16384] per core.
    nAT = -A.T, ntgt = -target: pure constant preprocessing (O(d^2)).
    """
    s = np.asarray(state, dtype=np.float32).reshape(NCORES, ROWS_PER_CORE, 4 * DIM)
    xew = np.empty((NCORES, 3, DIM, ROWS_PER_CORE), dtype=NP_BF16)
    xew[:, 0] = s[:, :, 0:DIM].transpose(0, 2, 1)
    xew[:, 1] = s[:, :, DIM:2 * DIM].transpose(0, 2, 1)
    xew[:, 2] = s[:, :, 3 * DIM:4 * DIM].transpose(0, 2, 1)

    nat = np.ascontiguousarray((-np.asarray(A, dtype=np.float32).T)).astype(NP_BF16)
    ntg = (-np.asarray(target, dtype=np.float32))[:, None]
    return {
        "xew": xew.reshape(NCORES * 3 * DIM, ROWS_PER_CORE),
        "nAT": np.concatenate([nat] * NCORES, axis=0),
        "ntgt": np.ascontiguousarray(np.concatenate([ntg] * NCORES, axis=0)),
    }


def run_on_device(state, A, target, repeat=1, n_timed=0, **buildkw):
    """Execute; optionally time n_timed extra calls (device-resident inputs).

    Returns (dxT_global [8*100, 16384] bf16, times_s list).
    """
    import jax
    from jax.sharding import NamedSharding, PartitionSpec
    import time

    runner = _get_runner(repeat, **buildkw)
    fn = runner["fn"]
    mesh = runner["mesh"]
    shard = NamedSharding(mesh, PartitionSpec("core"))

    cat = _concat_inputs(state, A, target)
    dev_in = [jax.device_put(cat[name], shard) for name in runner["in_names"]]
    dev_z = [
        jax.device_put(np.zeros((NCORES * sh[0], *sh[1:]), dt), shard)
        for (sh, dt) in runner["zero_shapes"]
    ]
    jax.block_until_ready(dev_z)

    outs = fn(*dev_in, *dev_z)
    jax.block_until_ready(outs)
    times = []
    for _ in range(n_timed):
        t0 = time.perf_counter()
        o = fn(*dev_in, *dev_z)
        jax.block_until_ready(o)
        times.append(time.perf_counter() - t0)
    result = np.asarray(outs[0])
    return result, times


def kernel(state, A, target):
    state = np.ascontiguousarray(np.asarray(state, dtype=np.float32))
    A = np.ascontiguousarray(np.asarray(A, dtype=np.float32))
    target = np.ascontiguousarray(np.asarray(target, dtype=np.float32))
    assert state.shape == (BATCH, 4 * DIM)

    dxt, _ = run_on_device(state, A, target, repeat=1)
    # dxt: [8*100, 16384] bf16 = per-core dx^T
    dx = (
        dxt.reshape(NCORES, DIM, ROWS_PER_CORE)
        .transpose(0, 2, 1)
        .reshape(BATCH, DIM)
        .astype(np.float32)
    )
    full = np.zeros((BATCH, 4 * DIM), dtype=np.float32)
    full[:, 0:DIM] = dx
    full[:, DIM:2 * DIM] = -dx
    return full


# revision 16
# speedup vs baseline: 2.1514x; 1.2897x over previous
"""Trainium2 Bass kernel for nn_CA_event (CA_event.forward batched ODE RHS).

reference:
    x   = state[:, 0:100]
    e_x = state[:, 100:200]
    W_a = state[:, 300:400]          (W_c = state[:, 200:300] unused)
    u   = W_a * (x + e_x - target)
    s   = x^2 / (1 + x^2)
    dx  = -x + s @ A.T + u * s
    out = concat([dx, -dx, 0, 0], axis=-1)      # [B, 400]

Strategy: pure data parallel over 8 NeuronCores (batch 131072 -> 16384
rows/core), FEATURE-MAJOR bf16 layout.  Host packs the three used state
slices transposed into one bf16 tensor xew[c] = [3, 100, 16384]
(planes x / e_x / W_a; feature dim on partitions), sends the constants
nAT = -A.T (bf16) and ntgt = -target ([100,1] f32).  The rel-err gate is
2e-2; bf16 I/O keeps the L2 error ~3e-3 while cutting HBM traffic from
2000 B/row (f32, both dx and -dx stored) to 800 B/row: reads 600 B
(x,e,W_a bf16) + writes 200 B (dx bf16 only; -dx is mirrored host-side,
the W_c/W_a derivative halves are structurally zero for any input).

Per 2048-row tile [100 partitions x 2048]:
    V:   he2 = x + e                     (bf16 tensor_tensor, 2x mode)
         rm1 = 1/(1+x^2) - 1 = -s       (custom DVE op, 1x: NOT-seed + 1 NR)
         u   = hm * w                   (2x)
         t2  = u * rm1 = -u*s           (2x)
    ACT: hm  = he2 + (-target)          (Identity activation, per-partition bias)
         out = Copy(psum) -> bf16       (the only PSUM read)
    PE:  psum = nAT@rm1 + nI@t2 + nI@x = A@s + u*s - x = dx^T
         (512-col chunks; no transposes anywhere -- the feature-major
          layout makes the moving operands k-major natively)
    DMA: one packed 3-plane load (sync/SP HWDGE), store on GpSimd SWDGE.
"""

import os
import sys

try:
    import concourse  # noqa: F401  (resolves via the environment's default path)
except ImportError:  # fall back for bare environments
    sys.path.insert(0, "/opt/trn_rl_repo")

import numpy as np
import ml_dtypes

import concourse.bass as bass
import concourse.bacc as bacc
import concourse.mybir as mybir
from concourse import tile
from concourse import masks

DIM = 100
BATCH = 131072
NCORES = 8
ROWS_PER_CORE = BATCH // NCORES          # 16384

F32 = mybir.dt.float32
BF16 = mybir.dt.bfloat16
NP_BF16 = ml_dtypes.bfloat16

_RUNNERS = {}  # key -> runner dict
_CA_OPS = None


def _register_ca_ops():
    """Register the fused custom-DVE op rm1 = 1/(1+x^2) - 1 (= -s) from x.

    Chebyshev bitwise-NOT reciprocal seed + one Newton pass on d = 1+x^2,
    minus 1; ~1e-3 rel accuracy on r, which is plenty under the 2e-2 gate.
    Same body as the baseline's proven CA_RM1_NR1.
    """
    global _CA_OPS
    if _CA_OPS is not None:
        return _CA_OPS
    from concourse import dve_ops
    from concourse.dve_spec import Spec, Src0, C0, C1, One, Bin, AluOp, sq
    from concourse.dve_uop import DveOpSpec

    dC = sq(Src0) + One
    ndC = Bin(AluOp.BITWISE_NOT, dC, dC)
    y0C = ndC * C0
    bodyC = y0C * (C1 - dC * y0C) - One

    def refC(in0, in1, s0, s1, imm2):
        d = (1.0 + in0.astype(np.float32) * in0).astype(np.float32)
        nd = (~d.view(np.int32)).view(np.float32)
        yy0 = (nd * np.float32(s0)).astype(np.float32)
        return (yy0 * (np.float32(s1) - d * yy0) - 1.0).astype(np.float32)

    name, spec = "CA_RM1_NR1", Spec(body=bodyC, reference=refC)
    if name not in dve_ops._SUB_OPCODE_FOR_NAME:
        row = max(dve_ops._SUB_OPCODE_FOR_NAME.values()) + 1
        assert row < 0x20
        dve_ops._SUB_OPCODE_FOR_NAME[name] = row
    shas = {}
    for ver in ("v3", "v4"):
        s = DveOpSpec(
            name=name,
            opcode=dve_ops.get_dve_sub_opcode(name),
            uops=dve_ops.lower(spec, ver=ver),
            rd1_en=dve_ops.has_src1(spec),
        )
        shas[ver] = s.sha(ver)
    op = dve_ops.DveOp(name, spec, subdim=False, uops_sha=shas)
    if not any(o.name == name for o in dve_ops.OPS):
        dve_ops.OPS.append(op)
        dve_ops.CUSTOM_DVE_SPECS[name] = spec
    _CA_OPS = (op,)
    return _CA_OPS


def _build(repeat=1, loop_k=1, t_rows=2048, he_eng="vector", hm_eng="sttx",
           xfold="v2", q_x="sync", q_e="scalar", q_w="gpsimd", q_st="gpsimd",
           unroll=3, bufs_in=4, bufs_work=4, bufs_out=4, pe_fuse=False,
           ablate=()):
    """Build the per-core Bacc module.

    he_eng: engine for he2 = x + e: 'vector' | 'pool'
    hm_eng: 'stt'  -> u = (he2 + ntgt) * w in one DVE scalar_tensor_tensor
            'act'  -> hm = Identity(he2 + ntgt) on ScalarE, u = hm*w on DVE
    xfold:  'v2'   -> v2 = t2 + x on DVE; psum = nAT@rm1 + nI@v2  (8 matmuls)
            'pe'   -> psum = nAT@rm1 + nI@t2 + nI@x               (12 matmuls)
    q_x/q_e/q_w/q_st: DMA queue for the x / e_x / W_a loads and the store:
            'sync' | 'scalar' | 'gpsimd' (SWDGE spreads over all 16 engines)
    unroll: passes per For_i iteration (amortizes the loop-boundary
            drain + semaphore reset, ~8us); loop_k must divide by it
    ablate: stages to skip for timing experiments only (output wrong):
            'dve', 'pe', 'act', 'load', 'store'
    """
    ablate = set(ablate)
    T = t_rows
    NTILES = ROWS_PER_CORE // T
    NCH = T // 512                     # matmul chunks per tile
    nc = bacc.Bacc("TRN2", target_bir_lowering=False, debug=False)

    xew = nc.declare_dram_parameter("xew", [3 * DIM, ROWS_PER_CORE], BF16, isOutput=False)
    nAT = nc.declare_dram_parameter("nAT", [DIM, DIM], BF16, isOutput=False)
    ntgt = nc.declare_dram_parameter("ntgt", [DIM, 1], F32, isOutput=False)
    out = nc.declare_dram_parameter("out", [DIM, ROWS_PER_CORE], BF16, isOutput=True)

    # [t][f, c, w]: feature f on partitions, plane c (x/e/w), row window w
    xew_t = xew.ap().rearrange("(c f) (t w) -> t f c w", c=3, w=T)
    out_t = out.ap().rearrange("f (t w) -> t f w", w=T)

    (op_rm1,) = _register_ca_ops()

    Q = {"sync": nc.sync, "scalar": nc.scalar, "gpsimd": nc.gpsimd}

    with tile.TileContext(nc) as tc:
        with (
            tc.tile_pool(name="consts", bufs=1) as consts,
            tc.tile_pool(name="inp", bufs=bufs_in) as inp,
            tc.tile_pool(name="work", bufs=bufs_work) as work,
            tc.tile_pool(name="outp", bufs=bufs_out) as outp,
            tc.tile_pool(name="psum_mm", bufs=2, space="PSUM") as psum_mm,
        ):
            # ---- one-time constants -------------------------------------
            nat_sb = consts.tile([DIM, DIM], BF16)
            nc.sync.dma_start(out=nat_sb[:], in_=nAT.ap())

            tgt_sb = consts.tile([DIM, 1], F32)
            nc.sync.dma_start(out=tgt_sb[:], in_=ntgt.ap())

            ident = consts.tile([DIM, DIM], F32)
            masks.make_identity(nc, ident[:])
            ni_sb = consts.tile([DIM, DIM], BF16)
            nc.scalar.mul(ni_sb[:], ident[:], -1.0)

            # ---- main loop ----------------------------------------------
            def emit_pass():
                for i in range(NTILES):
                    in_tile = inp.tile([DIM, 3, T], BF16, tag="in")
                    he2 = work.tile([DIM, T], BF16, tag="he")
                    if "load" not in ablate:
                        Q[q_x].dma_start(out=in_tile[:, 0, :],
                                         in_=xew_t[i][:, 0, :])
                        if he_eng == "dma":
                            # he2 = x + e computed by the SDMA inline ALU:
                            # load x into he2, then accumulate-load e onto it.
                            Q[q_e].dma_start(out=he2[:], in_=xew_t[i][:, 0, :])
                            Q[q_e].dma_start(out=he2[:], in_=xew_t[i][:, 1, :],
                                             accum_op=mybir.AluOpType.add)
                        else:
                            Q[q_e].dma_start(out=in_tile[:, 1, :],
                                             in_=xew_t[i][:, 1, :])
                        Q[q_w].dma_start(out=in_tile[:, 2, :],
                                         in_=xew_t[i][:, 2, :])
                    x = in_tile[:, 0, :]
                    e = in_tile[:, 1, :]
                    w = in_tile[:, 2, :]

                    skip_dve = "dve" in ablate

                    u = work.tile([DIM, T], BF16, tag="u")
                    rm1 = work.tile([DIM, T], BF16, tag="rm1")
                    t2 = work.tile([DIM, T], BF16, tag="t2")
                    mm = psum_mm.tile([DIM, T], F32, tag="mm")
                    if not skip_dve:
                        # rm1 first: it only needs x, and it unblocks the
                        # nAT matmuls early to keep PE fed.
                        nc.vector._custom_dve(
                            op_rm1, out=rm1[:], in0=x,
                            s0=float(np.float32(-0.23549792)),
                            s1=float(np.float32(2.0017324)),
                        )
                        if not pe_fuse and xfold != "actinit" and "pe" not in ablate:
                            for ci in range(NCH):
                                cs = slice(512 * ci, 512 * (ci + 1))
                                nc.tensor.matmul(mm[:, cs], nat_sb[:], rm1[:, cs],
                                                 start=True, stop=False,
                                                 skip_group_check=True)
                        if hm_eng == "sttx":
                            pass  # he2 not needed: fused into the stt below
                        elif he_eng == "pool":
                            nc.gpsimd.tensor_add(he2[:], x, e)
                        elif he_eng == "vector":
                            nc.vector.tensor_add(he2[:], x, e)
                        if hm_eng == "stt":
                            nc.vector.scalar_tensor_tensor(
                                u[:], he2[:], tgt_sb[:, 0:1], w,
                                op0=mybir.AluOpType.add,
                                op1=mybir.AluOpType.mult,
                            )
                        elif hm_eng == "sttx":
                            # hm = (x + ntgt) + e in ONE DVE op (no he2 op,
                            # no ScalarE hop)
                            hm = work.tile([DIM, T], BF16, tag="hm")
                            nc.vector.scalar_tensor_tensor(
                                hm[:], x, tgt_sb[:, 0:1], e,
                                op0=mybir.AluOpType.add,
                                op1=mybir.AluOpType.add,
                            )
                            nc.vector.tensor_mul(u[:], hm[:], w)
                        elif hm_eng == "ts":
                            # hm = he2 + ntgt via DVE tensor_scalar
                            # (per-partition scalar; 2x_2p/4x eligible)
                            hm = work.tile([DIM, T], BF16, tag="hm")
                            nc.vector.tensor_scalar_add(hm[:], he2[:],
                                                        tgt_sb[:, 0:1])
                            nc.vector.tensor_mul(u[:], hm[:], w)
                        else:
                            hm = work.tile([DIM, T], BF16, tag="hm")
                            nc.scalar.add(hm[:], he2[:], tgt_sb[:, 0:1])
                            nc.vector.tensor_mul(u[:], hm[:], w)
                        nc.vector.tensor_mul(t2[:], u[:], rm1[:])
                    else:
                        nc.vector.tensor_copy(rm1[:], x)
                        nc.vector.tensor_copy(t2[:], x)
                        if not pe_fuse:
                            for ci in range(NCH):
                                cs = slice(512 * ci, 512 * (ci + 1))
                                nc.tensor.matmul(mm[:, cs], nat_sb[:], rm1[:, cs],
                                                 start=True, stop=False,
                                                 skip_group_check=True)

                    if xfold == "actinit" and not skip_dve:
                        # psum := -x written by ScalarE; matmuls accumulate
                        # on top (no start=True reset!)
                        nc.scalar.mul(mm[:], x, -1.0)
                    if xfold == "v2" and not skip_dve:
                        v2 = work.tile([DIM, T], BF16, tag="v2")
                        nc.vector.tensor_add(v2[:], t2[:], x)
                    if "pe" not in ablate:
                        first = xfold != "actinit" or skip_dve
                        if pe_fuse or xfold == "actinit":
                            for ci in range(NCH):
                                cs = slice(512 * ci, 512 * (ci + 1))
                                nc.tensor.matmul(mm[:, cs], nat_sb[:], rm1[:, cs],
                                                 start=first, stop=False,
                                                 skip_group_check=True)
                        for ci in range(NCH):
                            cs = slice(512 * ci, 512 * (ci + 1))
                            if xfold == "actinit" and not skip_dve:
                                nc.tensor.matmul(mm[:, cs], ni_sb[:], t2[:, cs],
                                                 start=False, stop=True,
                                                 skip_group_check=True)
                            elif xfold == "v2" and not skip_dve:
                                nc.tensor.matmul(mm[:, cs], ni_sb[:], v2[:, cs],
                                                 start=False, stop=True,
                                                 skip_group_check=True)
                            else:
                                nc.tensor.matmul(mm[:, cs], ni_sb[:], t2[:, cs],
                                                 start=False, stop=False,
                                                 skip_group_check=True)
                                nc.tensor.matmul(mm[:, cs], ni_sb[:], x[:, cs],
                                                 start=False, stop=True,
                                                 skip_group_check=True)
                    else:
                        nc.vector.tensor_copy(mm[:], t2[:])

                    out_tile = outp.tile([DIM, T], BF16, tag="out")
                    if "act" not in ablate:
                        nc.scalar.copy(out_tile[:], mm[:])
                    else:
                        nc.vector.tensor_copy(out_tile[:], rm1[:])

                    if "store" not in ablate:
                        Q[q_st].dma_start(out=out_t[i], in_=out_tile[:])

            if loop_k > 1:
                stag = bool(int(os.environ.get("CA_STAG", "0")))
                u_ = unroll if loop_k % unroll == 0 else 1
                if loop_k // u_ > 1:
                    with tc.For_i(0, loop_k // u_, 1, staggered_reset=stag):
                        for _ in range(u_):
                            emit_pass()
                else:
                    for _ in range(loop_k):
                        emit_pass()
            else:
                for _ in range(repeat):
                    emit_pass()

    nc.compile()
    return nc


def _make_runner(nc):
    """Cached jitted shard_map executor for a prebuilt Bacc module."""
    import jax
    from jax.experimental.shard_map import shard_map
    from jax.sharding import Mesh, PartitionSpec
    from concourse import bass2jax

    bass2jax.install_neuronx_cc_hook()

    partition_name = nc.partition_id_tensor.name if nc.partition_id_tensor else None
    in_names, out_names, out_avals, zero_shapes = [], [], [], []
    for alloc in nc.m.functions[0].allocations:
        if not isinstance(alloc, mybir.MemoryLocationSet):
            continue
        name = alloc.memorylocations[0].name
        if alloc.kind == "ExternalInput":
            if name != partition_name:
                in_names.append(name)
        elif alloc.kind == "ExternalOutput":
            out_names.append(name)
            shape = tuple(alloc.tensor_shape)
            dtype = mybir.dt.np(alloc.dtype)
            out_avals.append(jax.core.ShapedArray(shape, dtype))
            zero_shapes.append((shape, dtype))
    n_params = len(in_names)
    n_outs = len(out_names)
    bind_in_names = list(in_names) + list(out_names)
    if partition_name is not None:
        bind_in_names.append(partition_name)

    def _body(*args):
        operands = list(args)
        if partition_name is not None:
            operands.append(bass2jax.partition_id_tensor())
        outs = bass2jax._bass_exec_p.bind(
            *operands,
            out_avals=tuple(out_avals),
            in_names=tuple(bind_in_names),
            out_names=tuple(out_names),
            lowering_input_output_aliases=(),
            sim_require_finite=True,
            sim_require_nnan=True,
            nc=nc,
        )
        return tuple(outs)

    devices = jax.devices()[:NCORES]
    assert len(devices) == NCORES
    mesh = Mesh(np.asarray(devices), ("core",))
    in_specs = (PartitionSpec("core"),) * (n_params + n_outs)
    out_specs = (PartitionSpec("core"),) * n_outs
    sharded = jax.jit(
        shard_map(_body, mesh=mesh, in_specs=in_specs, out_specs=out_specs,
                  check_rep=False),
        keep_unused=True,
    )

    return {
        "fn": sharded,
        "mesh": mesh,
        "in_names": in_names,
        "out_names": out_names,
        "zero_shapes": zero_shapes,
        "n_params": n_params,
    }


def _get_runner(repeat=1, **buildkw):
    key = (repeat, tuple(sorted(buildkw.items())))
    if key not in _RUNNERS:
        _RUNNERS[key] = _make_runner(_build(repeat, **buildkw))
    return _RUNNERS[key]


def _concat_inputs(state, A, target):
    """Host-side pack: shard + transpose to feature-major bf16.

    xew[c] = [x_c^T; e_c^T; w_c^T] stacked as [3*100, 16384] per core.
    nAT = -A.T, ntgt = -target: pure constant preprocessing (O(d^2)).
    """
    s = np.asarray(state, dtype=np.float32).reshape(NCORES, ROWS_PER_CORE, 4 * DIM)
    xew = np.empty((NCORES, 3, DIM, ROWS_PER_CORE), dtype=NP_BF16)
    xew[:, 0] = s[:, :, 0:DIM].transpose(0, 2, 1)
    xew[:, 1] = s[:, :, DIM:2 * DIM].transpose(0, 2, 1)
    xew[:, 2] = s[:, :, 3 * DIM:4 * DIM].transpose(0, 2, 1)

    nat = np.ascontiguousarray((-np.asarray(A, dtype=np.float32).T)).astype(NP_BF16)
    ntg = (-np.asarray(target, dtype=np.float32))[:, None]
    return {
        "xew": xew.reshape(NCORES * 3 * DIM, ROWS_PER_CORE),
        "nAT": np.concatenate([nat] * NCORES, axis=0),
        "ntgt": np.ascontiguousarray(np.concatenate([ntg] * NCORES, axis=0)),
    }


def run_on_device(state, A, target, repeat=1, n_timed=0, **buildkw):
    """Execute; optionally time n_timed extra calls (device-resident inputs).

    Returns (dxT_global [8*100, 16384] bf16, times_s list).
    """
    import jax
    from jax.sharding import NamedSharding, PartitionSpec
    import time

    runner = _get_runner(repeat, **buildkw)
    fn = runner["fn"]
    mesh = runner["mesh"]
    shard = NamedSharding(mesh, PartitionSpec("core"))

    cat = _concat_inputs(state, A, target)
    dev_in = [jax.device_put(cat[name], shard) for name in runner["in_names"]]
    dev_z = [
        jax.device_put(np.zeros((NCORES * sh[0], *sh[1:]), dt), shard)
        for (sh, dt) in runner["zero_shapes"]
    ]
    jax.block_until_ready(dev_z)

    outs = fn(*dev_in, *dev_z)
    jax.block_until_ready(outs)
    times = []
    for _ in range(n_timed):
        t0 = time.perf_counter()
        o = fn(*dev_in, *dev_z)
        jax.block_until_ready(o)
        times.append(time.perf_counter() - t0)
    result = np.asarray(outs[0])
    return result, times


def kernel(state, A, target):
    state = np.ascontiguousarray(np.asarray(state, dtype=np.float32))
    A = np.ascontiguousarray(np.asarray(A, dtype=np.float32))
    target = np.ascontiguousarray(np.asarray(target, dtype=np.float32))
    assert state.shape == (BATCH, 4 * DIM)

    dxt, _ = run_on_device(state, A, target, repeat=1)
    # dxt: [8*100, 16384] bf16 = per-core dx^T
    dx = (
        dxt.reshape(NCORES, DIM, ROWS_PER_CORE)
        .transpose(0, 2, 1)
        .reshape(BATCH, DIM)
        .astype(np.float32)
    )
    full = np.zeros((BATCH, 4 * DIM), dtype=np.float32)
    full[:, 0:DIM] = dx
    full[:, DIM:2 * DIM] = -dx
    return full


# revision 25
# speedup vs baseline: 2.3471x; 1.0910x over previous
"""Trainium2 Bass kernel for nn_CA_event (CA_event.forward batched ODE RHS).

reference:
    x   = state[:, 0:100]
    e_x = state[:, 100:200]
    W_a = state[:, 300:400]          (W_c = state[:, 200:300] unused)
    u   = W_a * (x + e_x - target)
    s   = x^2 / (1 + x^2)
    dx  = -x + s @ A.T + u * s
    out = concat([dx, -dx, 0, 0], axis=-1)      # [B, 400]

Strategy: pure data parallel over 8 NeuronCores (batch 131072 -> 16384
rows/core), FEATURE-MAJOR bf16 layout.  Host packs the three used state
slices transposed into one bf16 tensor xew[c] = [3, 100, 16384]
(planes x / e_x / W_a; feature dim on partitions), sends the constants
nAT = -A.T (bf16) and ntgt = -target ([100,1] f32).  The rel-err gate is
2e-2; bf16 I/O keeps the L2 error ~3e-3 while cutting HBM traffic from
2000 B/row (f32, both dx and -dx stored) to 800 B/row: reads 600 B
(x,e,W_a bf16) + writes 200 B (dx bf16 only; -dx is mirrored host-side,
the W_c/W_a derivative halves are structurally zero for any input).

Per 2048-row tile [100 partitions x 2048]:
    V:   he2 = x + e                     (bf16 tensor_tensor, 2x mode)
         rm1 = 1/(1+x^2) - 1 = -s       (custom DVE op, 1x: NOT-seed + 1 NR)
         u   = hm * w                   (2x)
         t2  = u * rm1 = -u*s           (2x)
    ACT: hm  = he2 + (-target)          (Identity activation, per-partition bias)
         out = Copy(psum) -> bf16       (the only PSUM read)
    PE:  psum = nAT@rm1 + nI@t2 + nI@x = A@s + u*s - x = dx^T
         (512-col chunks; no transposes anywhere -- the feature-major
          layout makes the moving operands k-major natively)
    DMA: one packed 3-plane load (sync/SP HWDGE), store on GpSimd SWDGE.
"""

import os
import sys

try:
    import concourse  # noqa: F401  (resolves via the environment's default path)
except ImportError:  # fall back for bare environments
    sys.path.insert(0, "/opt/trn_rl_repo")

import numpy as np
import ml_dtypes

import concourse.bass as bass
import concourse.bacc as bacc
import concourse.mybir as mybir
from concourse import tile
from concourse import masks

DIM = 100
BATCH = 131072
NCORES = 8
ROWS_PER_CORE = BATCH // NCORES          # 16384

F32 = mybir.dt.float32
BF16 = mybir.dt.bfloat16
NP_BF16 = ml_dtypes.bfloat16

_RUNNERS = {}  # key -> runner dict
_CA_OPS = None


def _register_ca_ops():
    """Register the fused custom-DVE op rm1 = 1/(1+x^2) - 1 (= -s) from x.

    Chebyshev bitwise-NOT reciprocal seed + one Newton pass on d = 1+x^2,
    minus 1; ~1e-3 rel accuracy on r, which is plenty under the 2e-2 gate.
    Same body as the baseline's proven CA_RM1_NR1.
    """
    global _CA_OPS
    if _CA_OPS is not None:
        return _CA_OPS
    from concourse import dve_ops
    from concourse.dve_spec import Spec, Src0, C0, C1, One, Bin, AluOp, sq
    from concourse.dve_uop import DveOpSpec

    dC = sq(Src0) + One
    ndC = Bin(AluOp.BITWISE_NOT, dC, dC)
    y0C = ndC * C0
    bodyC = y0C * (C1 - dC * y0C) - One

    def refC(in0, in1, s0, s1, imm2):
        d = (1.0 + in0.astype(np.float32) * in0).astype(np.float32)
        nd = (~d.view(np.int32)).view(np.float32)
        yy0 = (nd * np.float32(s0)).astype(np.float32)
        return (yy0 * (np.float32(s1) - d * yy0) - 1.0).astype(np.float32)

    name, spec = "CA_RM1_NR1", Spec(body=bodyC, reference=refC)
    if name not in dve_ops._SUB_OPCODE_FOR_NAME:
        row = max(dve_ops._SUB_OPCODE_FOR_NAME.values()) + 1
        assert row < 0x20
        dve_ops._SUB_OPCODE_FOR_NAME[name] = row
    shas = {}
    for ver in ("v3", "v4"):
        s = DveOpSpec(
            name=name,
            opcode=dve_ops.get_dve_sub_opcode(name),
            uops=dve_ops.lower(spec, ver=ver),
            rd1_en=dve_ops.has_src1(spec),
        )
        shas[ver] = s.sha(ver)
    op = dve_ops.DveOp(name, spec, subdim=False, uops_sha=shas)
    if not any(o.name == name for o in dve_ops.OPS):
        dve_ops.OPS.append(op)
        dve_ops.CUSTOM_DVE_SPECS[name] = spec
    _CA_OPS = (op,)
    return _CA_OPS


def _build(repeat=1, loop_k=1, t_rows=2048, he_eng="vector", hm_eng="ts",
           xfold="v2", q_x="sync", q_e="scalar", q_w="gpsimd", q_st="gpsimd",
           unroll=3, bufs_in=4, bufs_work=4, bufs_out=4, pe_fuse=False,
           psum_grain=None, ablate=()):
    """Build the per-core Bacc module.

    he_eng: engine for he2 = x + e: 'vector' | 'pool'
    hm_eng: 'stt'  -> u = (he2 + ntgt) * w in one DVE scalar_tensor_tensor
            'act'  -> hm = Identity(he2 + ntgt) on ScalarE, u = hm*w on DVE
    xfold:  'v2'   -> v2 = t2 + x on DVE; psum = nAT@rm1 + nI@v2  (8 matmuls)
            'pe'   -> psum = nAT@rm1 + nI@t2 + nI@x               (12 matmuls)
    q_x/q_e/q_w/q_st: DMA queue for the x / e_x / W_a loads and the store:
            'sync' | 'scalar' | 'gpsimd' (SWDGE spreads over all 16 engines)
    unroll: passes per For_i iteration (amortizes the loop-boundary
            drain + semaphore reset, ~8us); loop_k must divide by it
    ablate: stages to skip for timing experiments only (output wrong):
            'dve', 'pe', 'act', 'load', 'store'
    """
    ablate = set(ablate)
    T = t_rows
    NTILES = ROWS_PER_CORE // T
    NCH = T // 512                     # matmul chunks per tile
    if psum_grain is None:
        psum_grain = 1024 if xfold == "actinit" else T
    PH = psum_grain
    psum_bufs = max(1, (16384 // (4 * PH)))   # use all 8 PSUM banks
    nc = bacc.Bacc("TRN2", target_bir_lowering=False, debug=False)

    xew = nc.declare_dram_parameter("xew", [3 * DIM, ROWS_PER_CORE], BF16, isOutput=False)
    nAT = nc.declare_dram_parameter("nAT", [DIM, DIM], BF16, isOutput=False)
    ntgt = nc.declare_dram_parameter("ntgt", [DIM, 1], F32, isOutput=False)
    out = nc.declare_dram_parameter("out", [DIM, ROWS_PER_CORE], BF16, isOutput=True)

    # [t][f, c, w]: feature f on partitions, plane c (x/e/w), row window w
    xew_t = xew.ap().rearrange("(c f) (t w) -> t f c w", c=3, w=T)
    out_t = out.ap().rearrange("f (t w) -> t f w", w=T)

    (op_rm1,) = _register_ca_ops()

    Q = {"sync": nc.sync, "scalar": nc.scalar, "gpsimd": nc.gpsimd}

    with tile.TileContext(nc) as tc:
        with (
            tc.tile_pool(name="consts", bufs=1) as consts,
            tc.tile_pool(name="inp", bufs=bufs_in) as inp,
            tc.tile_pool(name="work", bufs=bufs_work) as work,
            tc.tile_pool(name="outp", bufs=bufs_out) as outp,
            tc.tile_pool(name="psum_mm", bufs=psum_bufs, space="PSUM") as psum_mm,
        ):
            # ---- one-time constants -------------------------------------
            nat_sb = consts.tile([DIM, DIM], BF16)
            nc.sync.dma_start(out=nat_sb[:], in_=nAT.ap())

            tgt_sb = consts.tile([DIM, 1], F32)
            nc.sync.dma_start(out=tgt_sb[:], in_=ntgt.ap())

            ident = consts.tile([DIM, DIM], F32)
            masks.make_identity(nc, ident[:])
            ni_sb = consts.tile([DIM, DIM], BF16)
            nc.scalar.mul(ni_sb[:], ident[:], -1.0)

            # ---- main loop ----------------------------------------------
            def emit_pass():
                for i in range(NTILES):
                    in_tile = inp.tile([DIM, 3, T], BF16, tag="in")
                    he2 = work.tile([DIM, T], BF16, tag="he")
                    if "load" not in ablate:
                        Q[q_x].dma_start(out=in_tile[:, 0, :],
                                         in_=xew_t[i][:, 0, :])
                        if he_eng == "dma":
                            # he2 = x + e computed by the SDMA inline ALU:
                            # load x into he2, then accumulate-load e onto it.
                            Q[q_e].dma_start(out=he2[:], in_=xew_t[i][:, 0, :])
                            Q[q_e].dma_start(out=he2[:], in_=xew_t[i][:, 1, :],
                                             accum_op=mybir.AluOpType.add)
                        else:
                            Q[q_e].dma_start(out=in_tile[:, 1, :],
                                             in_=xew_t[i][:, 1, :])
                        Q[q_w].dma_start(out=in_tile[:, 2, :],
                                         in_=xew_t[i][:, 2, :])
                    x = in_tile[:, 0, :]
                    e = in_tile[:, 1, :]
                    w = in_tile[:, 2, :]

                    skip_dve = "dve" in ablate

                    u = work.tile([DIM, T], BF16, tag="u")
                    rm1 = work.tile([DIM, T], BF16, tag="rm1")
                    t2 = work.tile([DIM, T], BF16, tag="t2")
                    if PH == T:
                        mm = psum_mm.tile([DIM, T], F32, tag="mm", name="mm")
                    else:
                        mm = None
                    if not skip_dve:
                        # rm1 first: it only needs x, and it unblocks the
                        # nAT matmuls early to keep PE fed.
                        nc.vector._custom_dve(
                            op_rm1, out=rm1[:], in0=x,
                            s0=float(np.float32(-0.23549792)),
                            s1=float(np.float32(2.0017324)),
                        )
                        if hm_eng == "sttx":
                            pass  # he2 not needed: fused into the stt below
                        elif he_eng == "pool":
                            nc.gpsimd.tensor_add(he2[:], x, e)
                        elif he_eng == "vector":
                            nc.vector.tensor_add(he2[:], x, e)
                        if hm_eng == "stt":
                            nc.vector.scalar_tensor_tensor(
                                u[:], he2[:], tgt_sb[:, 0:1], w,
                                op0=mybir.AluOpType.add,
                                op1=mybir.AluOpType.mult,
                            )
                        elif hm_eng == "sttx":
                            # hm = (x + ntgt) + e in ONE DVE op (no he2 op,
                            # no ScalarE hop)
                            hm = work.tile([DIM, T], BF16, tag="hm")
                            nc.vector.scalar_tensor_tensor(
                                hm[:], x, tgt_sb[:, 0:1], e,
                                op0=mybir.AluOpType.add,
                                op1=mybir.AluOpType.add,
                            )
                            nc.vector.tensor_mul(u[:], hm[:], w)
                        elif hm_eng == "ts":
                            # hm = he2 + ntgt via DVE tensor_scalar
                            # (per-partition scalar; 2x_2p/4x eligible)
                            hm = work.tile([DIM, T], BF16, tag="hm")
                            nc.vector.tensor_scalar_add(hm[:], he2[:],
                                                        tgt_sb[:, 0:1])
                            nc.vector.tensor_mul(u[:], hm[:], w)
                        else:
                            hm = work.tile([DIM, T], BF16, tag="hm")
                            nc.scalar.add(hm[:], he2[:], tgt_sb[:, 0:1])
                            nc.vector.tensor_mul(u[:], hm[:], w)
                        nc.vector.tensor_mul(t2[:], u[:], rm1[:])
                    else:
                        nc.vector.tensor_copy(rm1[:], x)
                        nc.vector.tensor_copy(t2[:], x)

                    if xfold == "v2" and not skip_dve:
                        v2 = work.tile([DIM, T], BF16, tag="v2")
                        nc.vector.tensor_add(v2[:], t2[:], x)

                    out_tile = outp.tile([DIM, T], BF16, tag="out")
                    for pi in range(T // PH):
                        ps = slice(PH * pi, PH * (pi + 1))
                        mmp = mm if PH == T else psum_mm.tile([DIM, PH], F32,
                                                              tag="mm")
                        if "pe" in ablate:
                            nc.vector.tensor_copy(mmp[:], t2[:, ps])
                        else:
                            ai = xfold == "actinit" and not skip_dve
                            if ai:
                                # psum := -x by ScalarE; matmuls accumulate
                                # on top (no start=True reset)
                                nc.scalar.mul(mmp[:], x[:, ps], -1.0)
                            for ci in range(PH // 512):
                                cl = slice(512 * ci, 512 * (ci + 1))
                                cg = slice(PH * pi + 512 * ci,
                                           PH * pi + 512 * (ci + 1))
                                nc.tensor.matmul(mmp[:, cl], nat_sb[:],
                                                 rm1[:, cg],
                                                 start=not ai, stop=False,
                                                 skip_group_check=True)
                                if xfold == "v2" and not skip_dve:
                                    nc.tensor.matmul(mmp[:, cl], ni_sb[:],
                                                     v2[:, cg],
                                                     start=False, stop=True,
                                                     skip_group_check=True)
                                else:
                                    nc.tensor.matmul(mmp[:, cl], ni_sb[:],
                                                     t2[:, cg],
                                                     start=False, stop=ai,
                                                     skip_group_check=True)
                                    if not ai:
                                        nc.tensor.matmul(mmp[:, cl], ni_sb[:],
                                                         x[:, cg],
                                                         start=False, stop=True,
                                                         skip_group_check=True)
                        if "act" not in ablate:
                            nc.scalar.copy(out_tile[:, ps], mmp[:])
                        else:
                            nc.vector.tensor_copy(out_tile[:, ps], rm1[:, ps])

                    if "store" not in ablate:
                        Q[q_st].dma_start(out=out_t[i], in_=out_tile[:])

            if loop_k > 1:
                stag = bool(int(os.environ.get("CA_STAG", "0")))
                u_ = unroll if loop_k % unroll == 0 else 1
                if loop_k // u_ > 1:
                    with tc.For_i(0, loop_k // u_, 1, staggered_reset=stag):
                        for _ in range(u_):
                            emit_pass()
                else:
                    for _ in range(loop_k):
                        emit_pass()
            else:
                for _ in range(repeat):
                    emit_pass()

    nc.compile()
    return nc


def _make_runner(nc):
    """Cached jitted shard_map executor for a prebuilt Bacc module."""
    import jax
    from jax.experimental.shard_map import shard_map
    from jax.sharding import Mesh, PartitionSpec
    from concourse import bass2jax

    bass2jax.install_neuronx_cc_hook()

    partition_name = nc.partition_id_tensor.name if nc.partition_id_tensor else None
    in_names, out_names, out_avals, zero_shapes = [], [], [], []
    for alloc in nc.m.functions[0].allocations:
        if not isinstance(alloc, mybir.MemoryLocationSet):
            continue
        name = alloc.memorylocations[0].name
        if alloc.kind == "ExternalInput":
            if name != partition_name:
                in_names.append(name)
        elif alloc.kind == "ExternalOutput":
            out_names.append(name)
            shape = tuple(alloc.tensor_shape)
            dtype = mybir.dt.np(alloc.dtype)
            out_avals.append(jax.core.ShapedArray(shape, dtype))
            zero_shapes.append((shape, dtype))
    n_params = len(in_names)
    n_outs = len(out_names)
    bind_in_names = list(in_names) + list(out_names)
    if partition_name is not None:
        bind_in_names.append(partition_name)

    def _body(*args):
        operands = list(args)
        if partition_name is not None:
            operands.append(bass2jax.partition_id_tensor())
        outs = bass2jax._bass_exec_p.bind(
            *operands,
            out_avals=tuple(out_avals),
            in_names=tuple(bind_in_names),
            out_names=tuple(out_names),
            lowering_input_output_aliases=(),
            sim_require_finite=True,
            sim_require_nnan=True,
            nc=nc,
        )
        return tuple(outs)

    devices = jax.devices()[:NCORES]
    assert len(devices) == NCORES
    mesh = Mesh(np.asarray(devices), ("core",))
    in_specs = (PartitionSpec("core"),) * (n_params + n_outs)
    out_specs = (PartitionSpec("core"),) * n_outs
    sharded = jax.jit(
        shard_map(_body, mesh=mesh, in_specs=in_specs, out_specs=out_specs,
                  check_rep=False),
        keep_unused=True,
    )

    return {
        "fn": sharded,
        "mesh": mesh,
        "in_names": in_names,
        "out_names": out_names,
        "zero_shapes": zero_shapes,
        "n_params": n_params,
    }


def _get_runner(repeat=1, **buildkw):
    key = (repeat, tuple(sorted(buildkw.items())))
    if key not in _RUNNERS:
        _RUNNERS[key] = _make_runner(_build(repeat, **buildkw))
    return _RUNNERS[key]


def _concat_inputs(state, A, target):
    """Host-side pack: shard + transpose to feature-major bf16.

    xew[c] = [x_c^T; e_c^T; w_c^T] stacked as [3*100, 16384] per core.
    nAT = -A.T, ntgt = -target: pure constant preprocessing (O(d^2)).
    """
    s = np.asarray(state, dtype=np.float32).reshape(NCORES, ROWS_PER_CORE, 4 * DIM)
    xew = np.empty((NCORES, 3, DIM, ROWS_PER_CORE), dtype=NP_BF16)
    xew[:, 0] = s[:, :, 0:DIM].transpose(0, 2, 1)
    xew[:, 1] = s[:, :, DIM:2 * DIM].transpose(0, 2, 1)
    xew[:, 2] = s[:, :, 3 * DIM:4 * DIM].transpose(0, 2, 1)

    nat = np.ascontiguousarray((-np.asarray(A, dtype=np.float32).T)).astype(NP_BF16)
    ntg = (-np.asarray(target, dtype=np.float32))[:, None]
    return {
        "xew": xew.reshape(NCORES * 3 * DIM, ROWS_PER_CORE),
        "nAT": np.concatenate([nat] * NCORES, axis=0),
        "ntgt": np.ascontiguousarray(np.concatenate([ntg] * NCORES, axis=0)),
    }


def run_on_device(state, A, target, repeat=1, n_timed=0, **buildkw):
    """Execute; optionally time n_timed extra calls (device-resident inputs).

    Returns (dxT_global [8*100, 16384] bf16, times_s list).
    """
    import jax
    from jax.sharding import NamedSharding, PartitionSpec
    import time

    runner = _get_runner(repeat, **buildkw)
    fn = runner["fn"]
    mesh = runner["mesh"]
    shard = NamedSharding(mesh, PartitionSpec("core"))

    cat = _concat_inputs(state, A, target)
    dev_in = [jax.device_put(cat[name], shard) for name in runner["in_names"]]
    dev_z = [
        jax.device_put(np.zeros((NCORES * sh[0], *sh[1:]), dt), shard)
        for (sh, dt) in runner["zero_shapes"]
    ]
    jax.block_until_ready(dev_z)

    outs = fn(*dev_in, *dev_z)
    jax.block_until_ready(outs)
    times = []
    for _ in range(n_timed):
        t0 = time.perf_counter()
        o = fn(*dev_in, *dev_z)
        jax.block_until_ready(o)
        times.append(time.perf_counter() - t0)
    result = np.asarray(outs[0])
    return result, times


def kernel(state, A, target):
    state = np.ascontiguousarray(np.asarray(state, dtype=np.float32))
    A = np.ascontiguousarray(np.asarray(A, dtype=np.float32))
    target = np.ascontiguousarray(np.asarray(target, dtype=np.float32))
    assert state.shape == (BATCH, 4 * DIM)

    dxt, _ = run_on_device(state, A, target, repeat=1)
    # dxt: [8*100, 16384] bf16 = per-core dx^T
    dx = (
        dxt.reshape(NCORES, DIM, ROWS_PER_CORE)
        .transpose(0, 2, 1)
        .reshape(BATCH, DIM)
        .astype(np.float32)
    )
    full = np.zeros((BATCH, 4 * DIM), dtype=np.float32)
    full[:, 0:DIM] = dx
    full[:, DIM:2 * DIM] = -dx
    return full
